# revision 13
# baseline (speedup 1.0000x reference)
"""Trainium2 Bass kernel for nn_InvariantCrossAttention.

Math: the reference computes softmax(-(Q2_i + K2_j), axis=j) — but -Q2_i is
constant along the softmax axis, so it cancels. The attention row is the same
for every query i, hence context[b,i] is i-independent and the final mean over
N is a no-op:

    out[b] = sum_j exp(-K2[b,j]) * K2[b,j] / sum_j exp(-K2[b,j])
    K2[b,j] = (x[b,j] - mean_j x[b,:])^2,  x = all_atom_features[:, :, 0]

cdr3_features does not affect the output (for any input values). The kernel
computes the reduction above on-device. Sharding: the post-simplification
problem is 128KB of input and ~20 instructions, so every core runs the full
(replicated) computation and core 0's output is returned — any cross-core
split would put a collective (multi-us) on a sub-us critical path.

Layout: x viewed as [128 partitions, 256 cols]; partition p holds batch p//32
(32 partitions per batch, contiguous 1KB rows -> full DMA bandwidth).
Cross-partition per-batch reduce/broadcast are tiny PE matmuls against
memset-generated group masks. The input load is split across both HWDGE
rings (SP + Activation) so the two halves' completion latencies overlap, and
the per-batch sum consumes each half directly via PSUM accumulation.
"""

import os

import numpy as np

B = 4  # batch
M = 8192  # all_atom length (softmax axis)
P = 128  # SBUF partitions
COLS = B * M // P  # 256 elements per partition
PPB = P // B  # 32 partitions per batch
N_CORES = 8

_cache = {}
last_results = None  # BassKernelResults of the most recent run (for test.py)


def _build():
    import concourse.bacc as bacc
    import concourse.bass as bass
    import concourse.mybir as mybir
    import concourse.tile as tile

    f32 = mybir.dt.float32
    nc = bacc.Bacc("TRN2", target_bir_lowering=False, debug=False)

    x_dram = nc.dram_tensor("x", [P, COLS], f32, kind="ExternalInput")
    mask_dram = nc.dram_tensor("mask", [P, B], f32, kind="ExternalInput")
    nmaskT_dram = nc.dram_tensor("nmaskT", [B, P], f32, kind="ExternalInput")
    out_dram = nc.dram_tensor("out", [B, 1], f32, kind="ExternalOutput")

    with tile.TileContext(nc) as tc:
        with (
            tc.tile_pool(name="sbuf", bufs=1) as pool,
            tc.tile_pool(name="psum", bufs=1, space=bass.MemorySpace.PSUM) as psum,
        ):
            X = pool.tile([P, COLS], f32)
            mask = pool.tile([P, B], f32)
            nmaskT = pool.tile([B, P], f32)
            zb = pool.tile([P, 1], f32)

            nc.gpsimd.memset(zb[:], 0.0)

            # Input halves on the two HWDGE rings (SP + Activation) so their
            # completion receipts overlap; constants pipeline behind them.
            H = P // 2
            nc.sync.dma_start(X[0:H, :], x_dram[0:H, :])
            nc.scalar.dma_start(X[H:P, :], x_dram[H:P, :])
            nc.sync.dma_start(mask[:], mask_dram[:])
            nc.scalar.dma_start(nmaskT[:], nmaskT_dram[:])

            partial = pool.tile([P, 1], f32)
            nc.vector.reduce_sum(partial[:], X[:], axis=mybir.AxisListType.X)

            # Per-batch sums then negative-mean broadcast via tiny PE matmuls.
            S1 = psum.tile([B, 1], f32)
            nc.tensor.matmul(S1[:], mask[:], partial[:])
            s4 = pool.tile([B, 1], f32)
            nc.vector.tensor_copy(s4[:], S1[:])
            NM = psum.tile([P, 1], f32)
            nc.tensor.matmul(NM[:], nmaskT[:], s4[:])
            nm = pool.tile([P, 1], f32)
            nc.vector.tensor_copy(nm[:], NM[:])

            # K2 = (x - mean)^2; w = exp(-K2) with per-partition sum;
            # wk = w*K2 with per-partition sum; mask.T @ [s1|s2] -> [4,2].
            K2 = pool.tile([P, COLS], f32)
            nc.scalar.activation(
                K2[:], X[:], mybir.ActivationFunctionType.Square, bias=nm[:]
            )

            partials = pool.tile([P, 2], f32)
            w = pool.tile([P, COLS], f32)
            nc.scalar.activation(
                w[:],
                K2[:],
                mybir.ActivationFunctionType.Exp,
                bias=zb[:],
                scale=-1.0,
                accum_out=partials[:, 0:1],
            )

            wk = pool.tile([P, COLS], f32)
            nc.vector.scalar_tensor_tensor(
                wk[:],
                w[:],
                1.0,
                K2[:],
                op0=mybir.AluOpType.mult,
                op1=mybir.AluOpType.mult,
                accum_out=partials[:, 1:2],
            )

            S2 = psum.tile([B, 2], f32)
            nc.tensor.matmul(
                S2[:], mask[:], partials[:]
            )

            r = pool.tile([B, 1], f32)
            nc.vector.reciprocal(r[:], S2[:, 0:1])
            res = pool.tile([B, 1], f32)
            nc.vector.tensor_tensor(
                res[:], S2[:, 1:2], r[:], op=mybir.AluOpType.mult
            )

            nc.sync.dma_start(out_dram[:], res[:])

    nc.compile()
    return nc


def kernel(cdr3_features=None, all_atom_features=None, **_unused):
    from concourse.bass_utils import run_bass_kernel_spmd

    global last_results
    if "nc" not in _cache:
        _cache["nc"] = _build()
    nc = _cache["nc"]

    x = np.ascontiguousarray(np.asarray(all_atom_features, dtype=np.float32)).reshape(
        P, COLS
    )
    mask = np.zeros((P, B), np.float32)
    for b in range(B):
        mask[b * PPB : (b + 1) * PPB, b] = 1.0
    nmaskT = np.ascontiguousarray(mask.T * np.float32(-1.0 / M))
    in_map = {"x": x, "mask": mask, "nmaskT": nmaskT}

    trace = bool(os.environ.get("KERNEL_TRACE"))
    last_results = run_bass_kernel_spmd(
        nc, [in_map] * N_CORES, list(range(N_CORES)), trace=trace
    )
    out = np.asarray(last_results.results[0]["out"], dtype=np.float32)
    return out.reshape(B, 1)


# revision 14
# speedup vs baseline: 1.0595x; 1.0595x over previous
"""Trainium2 Bass kernel for nn_InvariantCrossAttention.

Math: the reference computes softmax(-(Q2_i + K2_j), axis=j) — but -Q2_i is
constant along the softmax axis, so it cancels. The attention row is the same
for every query i, hence context[b,i] is i-independent and the final mean over
N is a no-op:

    out[b] = sum_j exp(-K2[b,j]) * K2[b,j] / sum_j exp(-K2[b,j])
    K2[b,j] = (x[b,j] - mean_j x[b,:])^2,  x = all_atom_features[:, :, 0]

cdr3_features does not affect the output (for any input values). The kernel
computes the reduction above on-device. Sharding: the post-simplification
problem is 128KB of input and ~20 instructions, so every core runs the full
(replicated) computation and core 0's output is returned — any cross-core
split would put a collective (multi-us) on a sub-us critical path.

Layout: x viewed as [128 partitions, 256 cols]; partition p holds batch p//32
(32 partitions per batch, contiguous 1KB rows -> full DMA bandwidth).
Cross-partition per-batch reduce/broadcast are tiny PE matmuls against
memset-generated group masks. The input load is split across both HWDGE
rings (SP + Activation) so the two halves' completion latencies overlap, and
the per-batch sum consumes each half directly via PSUM accumulation.
"""

import os

import numpy as np

B = 4  # batch
M = 8192  # all_atom length (softmax axis)
P = 128  # SBUF partitions
COLS = B * M // P  # 256 elements per partition
PPB = P // B  # 32 partitions per batch
N_CORES = 8

_cache = {}
last_results = None  # BassKernelResults of the most recent run (for test.py)


def _build():
    import concourse.bacc as bacc
    import concourse.bass as bass
    import concourse.mybir as mybir
    import concourse.tile as tile

    f32 = mybir.dt.float32
    nc = bacc.Bacc("TRN2", target_bir_lowering=False, debug=False)

    x_dram = nc.dram_tensor("x", [P, COLS], f32, kind="ExternalInput")
    nmaskT_dram = nc.dram_tensor("nmaskT", [B, P], f32, kind="ExternalInput")
    out_dram = nc.dram_tensor("out", [B, 1], f32, kind="ExternalOutput")

    with tile.TileContext(nc) as tc:
        with (
            tc.tile_pool(name="sbuf", bufs=1) as pool,
            tc.tile_pool(name="psum", bufs=1, space=bass.MemorySpace.PSUM) as psum,
        ):
            X = pool.tile([P, COLS], f32)
            mask = pool.tile([P, B], f32)
            nmaskT = pool.tile([B, P], f32)
            zb = pool.tile([P, 1], f32)

            nc.gpsimd.memset(zb[:], 0.0)
            # mask[p,b] = 1 iff p//32 == b, built with quadrant-aligned
            # memsets so no DMA shares the rings with the input load.
            nc.gpsimd.memset(mask[:], 0.0)
            for b in range(B):
                nc.gpsimd.memset(mask[b * PPB : (b + 1) * PPB, b : b + 1], 1.0)

            # Input halves get the two HWDGE rings (SP + Activation)
            # exclusively: any other traffic delays their completion sems.
            H = P // 2
            nc.sync.dma_start(X[0:H, :], x_dram[0:H, :])
            nc.scalar.dma_start(X[H:P, :], x_dram[H:P, :])
            # nmaskT (not memset-buildable: partition offsets 1..3 are not
            # quadrant-aligned) takes the SWDGE path instead.
            nc.gpsimd.dma_start(nmaskT[:], nmaskT_dram[:])

            partial = pool.tile([P, 1], f32)
            nc.vector.reduce_sum(partial[:], X[:], axis=mybir.AxisListType.X)

            # Per-batch sums then negative-mean broadcast via tiny PE matmuls.
            S1 = psum.tile([B, 1], f32)
            nc.tensor.matmul(S1[:], mask[:], partial[:])
            s4 = pool.tile([B, 1], f32)
            nc.vector.tensor_copy(s4[:], S1[:])
            NM = psum.tile([P, 1], f32)
            nc.tensor.matmul(NM[:], nmaskT[:], s4[:])
            nm = pool.tile([P, 1], f32)
            nc.vector.tensor_copy(nm[:], NM[:])

            # K2 = (x - mean)^2; w = exp(-K2) with per-partition sum;
            # wk = w*K2 with per-partition sum; mask.T @ [s1|s2] -> [4,2].
            K2 = pool.tile([P, COLS], f32)
            nc.scalar.activation(
                K2[:], X[:], mybir.ActivationFunctionType.Square, bias=nm[:]
            )

            partials = pool.tile([P, 2], f32)
            w = pool.tile([P, COLS], f32)
            nc.scalar.activation(
                w[:],
                K2[:],
                mybir.ActivationFunctionType.Exp,
                bias=zb[:],
                scale=-1.0,
                accum_out=partials[:, 0:1],
            )

            wk = pool.tile([P, COLS], f32)
            nc.vector.scalar_tensor_tensor(
                wk[:],
                w[:],
                1.0,
                K2[:],
                op0=mybir.AluOpType.mult,
                op1=mybir.AluOpType.mult,
                accum_out=partials[:, 1:2],
            )

            S2 = psum.tile([B, 2], f32)
            nc.tensor.matmul(
                S2[:], mask[:], partials[:]
            )

            r = pool.tile([B, 1], f32)
            nc.vector.reciprocal(r[:], S2[:, 0:1])
            res = pool.tile([B, 1], f32)
            nc.vector.tensor_tensor(
                res[:], S2[:, 1:2], r[:], op=mybir.AluOpType.mult
            )

            nc.sync.dma_start(out_dram[:], res[:])

    nc.compile()
    return nc


def kernel(cdr3_features=None, all_atom_features=None, **_unused):
    from concourse.bass_utils import run_bass_kernel_spmd

    global last_results
    if "nc" not in _cache:
        _cache["nc"] = _build()
    nc = _cache["nc"]

    x = np.ascontiguousarray(np.asarray(all_atom_features, dtype=np.float32)).reshape(
        P, COLS
    )
    nmaskT = np.zeros((B, P), np.float32)
    for b in range(B):
        nmaskT[b, b * PPB : (b + 1) * PPB] = np.float32(-1.0 / M)
    in_map = {"x": x, "nmaskT": nmaskT}

    trace = bool(os.environ.get("KERNEL_TRACE"))
    last_results = run_bass_kernel_spmd(
        nc, [in_map] * N_CORES, list(range(N_CORES)), trace=trace
    )
    out = np.asarray(last_results.results[0]["out"], dtype=np.float32)
    return out.reshape(B, 1)


# revision 15
# speedup vs baseline: 1.1249x; 1.0617x over previous
"""Trainium2 Bass kernel for nn_InvariantCrossAttention.

Math: the reference computes softmax(-(Q2_i + K2_j), axis=j) — but -Q2_i is
constant along the softmax axis, so it cancels. The attention row is the same
for every query i, hence context[b,i] is i-independent and the final mean over
N is a no-op:

    out[b] = sum_j exp(-K2[b,j]) * K2[b,j] / sum_j exp(-K2[b,j])
    K2[b,j] = (x[b,j] - mean_j x[b,:])^2,  x = all_atom_features[:, :, 0]

cdr3_features does not affect the output (for any input values). The kernel
computes the reduction above on-device. Sharding: the post-simplification
problem is 128KB of input and ~20 instructions, so every core runs the full
(replicated) computation and core 0's output is returned — any cross-core
split would put a collective (multi-us) on a sub-us critical path.

Layout: x viewed as [128 partitions, 256 cols]; partition p holds batch p//32
(32 partitions per batch, contiguous 1KB rows -> full DMA bandwidth).
Cross-partition per-batch reduce/broadcast are tiny PE matmuls against
memset-generated group masks. The input load is split across both HWDGE
rings (SP + Activation) so the two halves' completion latencies overlap, and
the per-batch sum consumes each half directly via PSUM accumulation.
"""

import os

import numpy as np

B = 4  # batch
M = 8192  # all_atom length (softmax axis)
P = 128  # SBUF partitions
COLS = B * M // P  # 256 elements per partition
PPB = P // B  # 32 partitions per batch
N_CORES = 8

_cache = {}
last_results = None  # BassKernelResults of the most recent run (for test.py)


def _build():
    import concourse.bacc as bacc
    import concourse.bass as bass
    import concourse.mybir as mybir
    import concourse.tile as tile

    f32 = mybir.dt.float32
    bf16 = mybir.dt.bfloat16
    nc = bacc.Bacc("TRN2", target_bir_lowering=False, debug=False)

    x_dram = nc.dram_tensor("x", [P, COLS], f32, kind="ExternalInput")
    nmaskT_dram = nc.dram_tensor("nmaskT", [B, P], bf16, kind="ExternalInput")
    out_dram = nc.dram_tensor("out", [B, 1], f32, kind="ExternalOutput")

    with tile.TileContext(nc) as tc:
        with (
            tc.tile_pool(name="sbuf", bufs=1) as pool,
            tc.tile_pool(name="psum", bufs=1, space=bass.MemorySpace.PSUM) as psum,
        ):
            X = pool.tile([P, COLS], f32)
            mask = pool.tile([P, B], f32)
            nmaskT = pool.tile([B, P], bf16)
            zb = pool.tile([P, 1], f32)

            nc.gpsimd.memset(zb[:], 0.0)
            # mask[p,b] = 1 iff p//32 == b, built with quadrant-aligned
            # memsets so no constant DMA delays the input load's sems.
            nc.vector.memset(mask[:], 0.0)
            for b in range(B):
                nc.vector.memset(mask[b * PPB : (b + 1) * PPB, b : b + 1], 1.0)

            # Input halves get the two HWDGE rings (SP + Activation) first;
            # nmaskT (not memset-buildable: partition offsets 1..3 are not
            # quadrant-aligned) pipelines behind X_h1 on the Scalar ring.
            H = P // 2
            nc.sync.dma_start(X[0:H, :], x_dram[0:H, :])
            nc.scalar.dma_start(X[H:P, :], x_dram[H:P, :])
            nc.scalar.dma_start(nmaskT[:], nmaskT_dram[:])

            partial = pool.tile([P, 1], f32)
            nc.vector.reduce_sum(partial[:], X[:], axis=mybir.AxisListType.X)

            # Per-batch sums then negative-mean broadcast via tiny PE matmuls.
            S1 = psum.tile([B, 1], f32)
            nc.tensor.matmul(S1[:], mask[:], partial[:])
            s4 = pool.tile([B, 1], bf16)
            nc.vector.tensor_copy(s4[:], S1[:])
            NM = psum.tile([P, 1], f32)
            nc.tensor.matmul(NM[:], nmaskT[:], s4[:])
            nm = pool.tile([P, 1], f32)
            nc.vector.tensor_copy(nm[:], NM[:])

            # K2 = (x - mean)^2; w = exp(-K2) with per-partition sum;
            # wk = w*K2 with per-partition sum; mask.T @ [s1|s2] -> [4,2].
            K2 = pool.tile([P, COLS], f32)
            nc.scalar.activation(
                K2[:], X[:], mybir.ActivationFunctionType.Square, bias=nm[:]
            )

            partials = pool.tile([P, 2], f32)
            w = pool.tile([P, COLS], f32)
            nc.scalar.activation(
                w[:],
                K2[:],
                mybir.ActivationFunctionType.Exp,
                bias=zb[:],
                scale=-1.0,
                accum_out=partials[:, 0:1],
            )

            wk = pool.tile([P, COLS], f32)
            nc.vector.scalar_tensor_tensor(
                wk[:],
                w[:],
                1.0,
                K2[:],
                op0=mybir.AluOpType.mult,
                op1=mybir.AluOpType.mult,
                accum_out=partials[:, 1:2],
            )

            S2 = psum.tile([B, 2], f32)
            nc.tensor.matmul(
                S2[:], mask[:], partials[:]
            )

            r = pool.tile([B, 1], f32)
            nc.vector.reciprocal(r[:], S2[:, 0:1])
            res = pool.tile([B, 1], f32)
            nc.vector.tensor_tensor(
                res[:], S2[:, 1:2], r[:], op=mybir.AluOpType.mult
            )

            nc.sync.dma_start(out_dram[:], res[:])

    nc.compile()
    return nc


def kernel(cdr3_features=None, all_atom_features=None, **_unused):
    from concourse.bass_utils import run_bass_kernel_spmd

    global last_results
    if "nc" not in _cache:
        _cache["nc"] = _build()
    nc = _cache["nc"]

    x = np.ascontiguousarray(np.asarray(all_atom_features, dtype=np.float32)).reshape(
        P, COLS
    )
    import ml_dtypes

    nmaskT = np.zeros((B, P), ml_dtypes.bfloat16)
    for b in range(B):
        nmaskT[b, b * PPB : (b + 1) * PPB] = ml_dtypes.bfloat16(-1.0 / M)
    in_map = {"x": x, "nmaskT": nmaskT}

    trace = bool(os.environ.get("KERNEL_TRACE"))
    last_results = run_bass_kernel_spmd(
        nc, [in_map] * N_CORES, list(range(N_CORES)), trace=trace
    )
    out = np.asarray(last_results.results[0]["out"], dtype=np.float32)
    return out.reshape(B, 1)
